# revision 9
# baseline (speedup 1.0000x reference)
"""Differential quadratic causal linear attention on 8 TRN2 NeuronCores.

Chunked linear-attention formulation (super-chunks of 256 rows):
  out_l = q12_l . (sum_{m<=l} k12_m (x) v_m),  q12 = [q1; -a*q2], k12 = [k1; k2]
  per l-chunk c (128 rows): intra-chunk quadratic blocks (tril-masked diag
  block + full block for odd chunks) + inter term q12_c . S_{<c//2} where
  S_j accumulates k12 (x) v over super-chunk j (PSUM f32, snapshots in bf16).
  The ones-column appended to v carries the denominator for free.

Shapes (hardcoded): B=4, H=16, L=1024, D=64; host pre-transposes q,k to
[d, L] bf16, pre-tiles v (+ones) to [128, 8, 65] bf16, W to [64, 128]
([W1 | W2]) bf16 per head; output comes back as [128, 8, 64] f32 tiles.
Sharding: batch*heads over 8 cores -> 8 (b,h) pairs per core.
"""

import numpy as np
import ml_dtypes

import concourse.bass as bass
import concourse.bacc as bacc
import concourse.mybir as mybir
import concourse.tile as tile
from concourse.bass_utils import run_bass_kernel_spmd

B, H, L, D = 4, 16, 1024, 64
NCORES = 8
HPC = H // NCORES          # heads per core
NP = B * HPC               # (b,h) pairs per core
NT = L // 128              # l-chunks of 128
EPS = 1e-6
F32 = mybir.dt.float32
F32R = mybir.dt.float32r
BF16 = mybir.dt.bfloat16
AF = mybir.ActivationFunctionType
OP = mybir.AluOpType
BF = ml_dtypes.bfloat16

_CACHE = {}


def _ident_np():
    return np.eye(128, dtype=BF)


def _maskbd_np():
    """[128, 512] f32 block-diag tril: col block j keeps m <= l within block."""
    m = np.arange(128)[:, None]
    l = np.arange(128)[None, :]
    blk = (m <= l).astype(np.float32)
    return np.tile(blk, (1, 4))


def _bcast(ap, n):
    """Append a stride-0 axis of length n to an AP (free-dim broadcast)."""
    return bass.AP(tensor=ap.tensor, offset=ap.offset, ap=list(ap.ap) + [[0, n]])


def _build(alpha: float, reps: int = 1, dbg: bool = False):
    nc = bacc.Bacc(trn_type="TRN2", target_bir_lowering=False, debug=False)

    qkt_d = nc.dram_tensor("qkt", [NP, 128, L], F32R, kind="ExternalInput").ap()
    v_d = nc.dram_tensor("v", [NP, 128, NT, D + 1], BF16, kind="ExternalInput").ap()
    wq_d = nc.dram_tensor("wq", [HPC, D, 128], F32R, kind="ExternalInput").ap()
    wk_d = nc.dram_tensor("wk", [HPC, D, 128], F32R, kind="ExternalInput").ap()
    out_d = nc.dram_tensor("out", [NP, 128, NT, D], F32, kind="ExternalOutput").ap()
    dbg_d = {}
    if dbg:
        for nm, shape, dt in [
            ("dA", [128, 2 * L], BF16), ("dk12n", [128, 6, 128], BF16),
            ("dPsA", [128, 512], BF16), ("dPsB", [128, 512], BF16),
            ("dPsF", [128, 512], BF16), ("dSsb", [128, 3, 65], BF16),
            ("dous", [128, NT, D + 1], F32),
            ("dden", [128, NT], F32), ("ddi", [128, NT], F32),
            ("doutf", [128, NT, D], F32),
        ]:
            dbg_d[nm] = nc.dram_tensor(nm, shape, dt,
                                       kind="ExternalOutput").ap()

    ident_d = nc.inline_tensor(_ident_np(), name="identbf").ap()
    maskbd_d = nc.inline_tensor(_maskbd_np(), name="maskbd").ap()

    with tile.TileContext(nc) as tc:
        with (
            tc.tile_pool(name="statics", bufs=1) as statics,
            tc.tile_pool(name="io", bufs=3) as io,
            tc.tile_pool(name="sb", bufs=3) as sb,
            tc.tile_pool(name="ps_fm", bufs=2, space="PSUM") as ps_fm,
            tc.tile_pool(name="ps_pp", bufs=2, space="PSUM") as ps_pp,
            tc.tile_pool(name="ps_tr", bufs=2, space="PSUM") as ps_tr,
            tc.tile_pool(name="ps_mx", bufs=2, space="PSUM") as ps_mx,
        ):
            ident = statics.tile([128, 128], BF16, tag="ident")
            nc.sync.dma_start(out=ident, in_=ident_d)
            maskbd = statics.tile([128, 512], F32, tag="maskbd")
            nc.sync.dma_start(out=maskbd, in_=maskbd_d)

            wq_t, wk_t = [], []
            for hl in range(HPC):
                t_wq = statics.tile([64, 128], F32R, tag=f"wq{hl}", name=f"wq{hl}")
                nc.sync.dma_start(out=t_wq, in_=wq_d[hl])
                wq_t.append(t_wq)
                t_wk = statics.tile([128, 128], F32R, tag=f"wk{hl}", name=f"wk{hl}")
                nc.sync.dma_start(out=t_wk[64:128, :], in_=wk_d[hl])
                wk_t.append(t_wk)

            # greedy DVE/ACT load balancer for PSUM->SBUF evacuations
            load = {"dve": 0.0, "act": 0.0}

            def evac(dst, src, cols, relu=False):
                cd = cols * 1.04 + 120.0 + load["dve"]
                ca = cols * 0.833 + 293.0 + load["act"]
                if cd <= ca:
                    load["dve"] = cd
                    if relu:
                        nc.vector.tensor_relu(dst, src)
                    else:
                        nc.vector.tensor_copy(dst, src)
                else:
                    load["act"] = ca
                    if relu:
                        nc.scalar.activation(dst, src, AF.Relu)
                    else:
                        nc.scalar.copy(dst, src)

            st = {}  # per-pair live tiles

            def stage_load(p):
                pd = p % NP
                qkT = io.tile([128, L], F32R, tag="qkT", name=f"qkT{p}")
                nc.sync.dma_start(out=qkT, in_=qkt_d[pd])
                vn = io.tile([128, NT, D + 1], BF16, tag="vn", name=f"vn{p}")
                nc.sync.dma_start(out=vn, in_=v_d[pd])
                st[p] = {"qkT": qkT, "vn": vn}

            def stage_fmap(p):
                hl = (p % NP) % HPC
                qkT = st[p]["qkT"]
                A = sb.tile([128, 2 * L], BF16, tag="A", name=f"A{p}")
                ad = sb.tile([128, 2 * L], BF16, tag="ad", name=f"ad{p}")
                tmp = sb.tile([128, 2 * L], BF16, tag="tmp", name=f"tmp{p}")
                for qk in range(2):
                    wm = wq_t[hl] if qk == 0 else wk_t[hl][64:128, :]
                    rows = slice(0, 64) if qk == 0 else slice(64, 128)
                    for lc in range(2):
                        fm = ps_fm.tile([128, 512], F32, tag="fm",
                                        name=f"fm{p}_{qk}_{lc}")
                        nc.tensor.matmul(fm, wm,
                                         qkT[rows, lc * 512:(lc + 1) * 512],
                                         start=True, stop=True)
                        cs = slice(qk * L + lc * 512, qk * L + (lc + 1) * 512)
                        # x1 -> A top; sigmoid(x2pre) -> tmp bottom
                        evac(A[0:64, cs], fm[0:64, :], 512, relu=True)
                        nc.scalar.activation(tmp[64:128, cs], fm[64:128, :],
                                             AF.Sigmoid)
                        nc.gpsimd.dma_start(out=ad[64:128, cs], in_=A[0:64, cs])
                        if qk == 0:
                            # q2 = -alpha * max(sig, 0.5) * q1
                            nc.vector.tensor_scalar(
                                tmp[64:128, cs], tmp[64:128, cs], 0.5, -alpha,
                                op0=OP.max, op1=OP.mult)
                            nc.vector.tensor_tensor(
                                A[64:128, cs], tmp[64:128, cs], ad[64:128, cs],
                                op=OP.mult)
                        else:
                            # k2 = max(sig, 0.5) * k1
                            nc.vector.scalar_tensor_tensor(
                                A[64:128, cs], tmp[64:128, cs], 0.5,
                                ad[64:128, cs], op0=OP.max, op1=OP.mult)
                st[p]["A"] = A

            def Q12(p, c):
                return st[p]["A"][:, c * 128:(c + 1) * 128]

            def K12(p, c):
                return st[p]["A"][:, L + c * 128:L + (c + 1) * 128]

            def stage_prep(p):
                A = st[p]["A"]
                vn = st[p]["vn"]
                # transposes of K12 chunks 0..5 -> k12nat (natural layout)
                k12n = sb.tile([128, 6, 128], BF16, tag="k12n", name=f"k12n{p}")
                trs = []
                for g in range(2):
                    tr = ps_tr.tile([128, 512], BF16, tag="tr",
                                    name=f"tr{p}_{g}")
                    n = 4 if g == 0 else 2
                    for j in range(n):
                        nc.tensor.transpose(tr[:, j * 128:(j + 1) * 128],
                                            K12(p, g * 4 + j), ident)
                    trs.append((tr, n))
                # intra-chunk quadratic blocks
                pTd = ps_pp.tile([128, 512], F32, tag="pp", name=f"pTd{p}")
                for c in range(4):
                    nc.tensor.matmul(pTd[:, c * 128:(c + 1) * 128],
                                     K12(p, c), Q12(p, c),
                                     start=True, stop=True,
                                     skip_group_check=True)
                # evac transposes while diag MMs run
                for g, (tr, n) in enumerate(trs):
                    evac(k12n[:, g * 4:g * 4 + n, :], tr[:, 0:n * 128], n * 128)
                pTd2 = ps_pp.tile([128, 512], F32, tag="pp", name=f"pTd2{p}")
                for c in range(4, 8):
                    nc.tensor.matmul(pTd2[:, (c - 4) * 128:(c - 3) * 128],
                                     K12(p, c), Q12(p, c),
                                     start=True, stop=True,
                                     skip_group_check=True)
                PsA = sb.tile([128, 512], BF16, tag="PsA", name=f"PsA{p}")
                nc.vector.tensor_tensor(PsA, pTd, maskbd, op=OP.mult)
                pTf = ps_pp.tile([128, 512], F32, tag="pp", name=f"pTf{p}")
                for j in range(4):
                    nc.tensor.matmul(pTf[:, j * 128:(j + 1) * 128],
                                     K12(p, 2 * j), Q12(p, 2 * j + 1),
                                     start=True, stop=True,
                                     skip_group_check=True)
                PsB = sb.tile([128, 512], BF16, tag="PsB", name=f"PsB{p}")
                nc.vector.tensor_tensor(PsB, pTd2, maskbd, op=OP.mult)
                # state deltas: sd[:, j, :] accumulates super-chunk j
                mx1 = ps_mx.tile([128, 512], F32, tag="mx", name=f"mx1{p}")
                sd = mx1[:, 260:455].rearrange("p (j d) -> p j d", j=3)
                for c in range(6):
                    j = c // 2
                    nc.tensor.matmul(sd[:, j, :], k12n[:, c, :], vn[:, c, :],
                                     start=(c % 2 == 0), stop=(c % 2 == 1),
                                     skip_group_check=True)
                PsF = sb.tile([128, 512], BF16, tag="PsF", name=f"PsF{p}")
                evac(PsF, pTf, 512)
                # S snapshots (bf16): S[0]=sd0; S[j]=S[j-1]+sd[j]
                Ssb = sb.tile([128, 3, 65], BF16, tag="Ssb", name=f"Ssb{p}")
                nc.vector.tensor_copy(Ssb[:, 0, :], sd[:, 0, :])
                for j in (1, 2):
                    nc.vector.tensor_tensor(Ssb[:, j, :], Ssb[:, j - 1, :],
                                            sd[:, j, :], op=OP.add)
                st[p].update(PsA=PsA, PsB=PsB, PsF=PsF, Ssb=Ssb, mx1=mx1,
                             k12n=k12n)

            def stage_out(p):
                pd = p % NP
                s = st[p]
                vn = s["vn"]
                ous = sb.tile([128, NT, D + 1], F32, tag="ous", name=f"ous{p}")
                for g in range(2):
                    if g == 0:
                        ou = s["mx1"][:, 0:260]
                    else:
                        mx2 = ps_mx.tile([128, 512], F32, tag="mx",
                                         name=f"mx2{p}")
                        ou = mx2[:, 0:260]
                    ou = ou.rearrange("p (t d) -> p t d", t=4)
                    for t in range(4):
                        c = g * 4 + t
                        Pst = (s["PsA"] if g == 0 else s["PsB"])
                        mms = [(Pst[:, t * 128:(t + 1) * 128], vn[:, c, :])]
                        if c % 2 == 1:
                            mms.append((s["PsF"][:, (c // 2) * 128:
                                                 (c // 2 + 1) * 128],
                                        vn[:, c - 1, :]))
                        if c >= 2:
                            mms.append((Q12(p, c), s["Ssb"][:, c // 2 - 1, :]))
                        for i, (lhsT, rhs) in enumerate(mms):
                            nc.tensor.matmul(ou[:, t, :], lhsT, rhs,
                                             start=(i == 0),
                                             stop=(i == len(mms) - 1),
                                             skip_group_check=True)
                    evac(ous[:, g * 4:(g + 1) * 4, :], ou, 260)
                if dbg and p == 0:
                    nc.sync.dma_start(out=dbg_d["dA"], in_=s["A"])
                    nc.sync.dma_start(out=dbg_d["dk12n"], in_=s["k12n"])
                    nc.sync.dma_start(out=dbg_d["dPsA"], in_=s["PsA"])
                    nc.sync.dma_start(out=dbg_d["dPsB"], in_=s["PsB"])
                    nc.sync.dma_start(out=dbg_d["dPsF"], in_=s["PsF"])
                    nc.sync.dma_start(out=dbg_d["dSsb"], in_=s["Ssb"])
                    nc.sync.dma_start(out=dbg_d["dous"], in_=ous)
                den = sb.tile([128, NT], F32, tag="den", name=f"den{p}")
                nc.vector.tensor_scalar(den, ous[:, :, 64], EPS, None,
                                        op0=OP.add)
                di = sb.tile([128, NT], F32, tag="di", name=f"di{p}")
                nc.vector.reciprocal(di, den)
                outf = io.tile([128, NT, D], F32, tag="outf", name=f"outf{p}")
                nc.vector.tensor_tensor(outf, ous[:, :, 0:64],
                                        _bcast(di[:, :], 64), op=OP.mult)
                if dbg and p == 0:
                    nc.sync.dma_start(out=dbg_d["dden"], in_=den)
                    nc.sync.dma_start(out=dbg_d["ddi"], in_=di)
                    nc.sync.dma_start(out=dbg_d["doutf"], in_=outf)
                nc.sync.dma_start(out=out_d[pd], in_=outf)
                del st[p]

            seq = list(range(reps * NP))
            n = len(seq)
            stage_load(seq[0])
            if n > 1:
                stage_load(seq[1])
            stage_fmap(seq[0])
            for i, p in enumerate(seq):
                stage_prep(p)
                if i + 1 < n:
                    stage_fmap(seq[i + 1])
                stage_out(p)
                if i + 2 < n:
                    stage_load(seq[i + 2])
    nc.compile()
    return nc


def _get_nc(alpha: float = 0.3, reps: int = 1):
    key = ("nc", float(alpha), reps)
    if key not in _CACHE:
        _CACHE[key] = _build(float(alpha), reps)
    return _CACHE[key]


def prepare_in_maps(inputs: dict) -> list[dict]:
    """Host-side shard + relayout: full inputs -> per-core in_maps."""
    q = np.ascontiguousarray(np.asarray(inputs["query_states"], np.float32))
    k = np.ascontiguousarray(np.asarray(inputs["key_states"], np.float32))
    v = np.ascontiguousarray(np.asarray(inputs["value_states"], np.float32))
    v = np.concatenate([v, np.ones(v.shape[:-1] + (1,), np.float32)], axis=-1)
    w1q = np.asarray(inputs["W1q"], np.float32)
    w1k = np.asarray(inputs["W1k"], np.float32)
    w2q = np.asarray(inputs["W2q"], np.float32)
    w2k = np.asarray(inputs["W2k"], np.float32)
    in_maps = []
    for c in range(NCORES):
        hs = slice(c * HPC, (c + 1) * HPC)
        qc = q[:, hs].reshape(NP, L, D).transpose(0, 2, 1)
        kc = k[:, hs].reshape(NP, L, D).transpose(0, 2, 1)
        qkt = np.ascontiguousarray(np.concatenate([qc, kc], axis=1))
        vc = v[:, hs].reshape(NP, NT, 128, D + 1).transpose(0, 2, 1, 3)
        vc = np.ascontiguousarray(vc).astype(BF)
        wq = np.ascontiguousarray(np.concatenate([w1q[hs], w2q[hs]], axis=2))
        wk = np.ascontiguousarray(np.concatenate([w1k[hs], w2k[hs]], axis=2))
        in_maps.append({"qkt": qkt, "v": vc, "wq": wq, "wk": wk})
    return in_maps


def finish_out(res_out: np.ndarray) -> np.ndarray:
    """Device-tiled [NP, 128, NT, D] -> [NP, L, D]."""
    return res_out.transpose(0, 2, 1, 3).reshape(NP, L, D)


def kernel(query_states, key_states, value_states, W1q, W1k, W2q, W2k, alpha):
    al = float(np.asarray(alpha, dtype=np.float32).reshape(-1)[0])
    inputs = {
        "query_states": query_states, "key_states": key_states,
        "value_states": value_states, "W1q": W1q, "W1k": W1k,
        "W2q": W2q, "W2k": W2k,
    }
    in_maps = prepare_in_maps(inputs)
    nc = _get_nc(al)
    res = run_bass_kernel_spmd(nc, in_maps, core_ids=list(range(NCORES)))
    out = np.empty((B, H, L, D), dtype=np.float32)
    for c in range(NCORES):
        o = finish_out(res.results[c]["out"]).reshape(B, HPC, L, D)
        out[:, c * HPC:(c + 1) * HPC] = o
    return out


# revision 17
# speedup vs baseline: 30.2291x; 30.2291x over previous
"""Differential quadratic causal linear attention on 8 TRN2 NeuronCores.

Chunked linear-attention formulation (super-chunks of 256 rows):
  out_l = q12_l . (sum_{m<=l} k12_m (x) v_m),  q12 = [q1; -a*q2], k12 = [k1; k2]
  per l-chunk c (128 rows): intra-chunk quadratic blocks (tril-masked diag
  block + full block for odd chunks) + inter term q12_c . S_{<c//2} where
  S_j accumulates k12 (x) v over super-chunk j (PSUM f32, snapshots in bf16).
  The ones-column appended to v carries the denominator for free.

Shapes (hardcoded): B=4, H=16, L=1024, D=64; host pre-transposes q,k to
[d, L] bf16, pre-tiles v (+ones) to [128, 8, 65] bf16, W to [64, 128]
([W1 | W2]) bf16 per head; output comes back as [128, 8, 64] f32 tiles.
Sharding: batch*heads over 8 cores -> 8 (b,h) pairs per core.
"""

import numpy as np
import ml_dtypes

import concourse.bass as bass
import concourse.bacc as bacc
import concourse.mybir as mybir
import concourse.tile as tile
from concourse.bass_utils import run_bass_kernel_spmd

B, H, L, D = 4, 16, 1024, 64
NCORES = 8
HPC = H // NCORES          # heads per core
NP = B * HPC               # (b,h) pairs per core
NT = L // 128              # l-chunks of 128
EPS = 1e-6
F32 = mybir.dt.float32
F32R = mybir.dt.float32r
BF16 = mybir.dt.bfloat16
AF = mybir.ActivationFunctionType
OP = mybir.AluOpType
BF = ml_dtypes.bfloat16

_CACHE = {}


def _ident_np():
    return np.eye(128, dtype=BF)


def _maskbd_np():
    """[128, 512] f32 block-diag tril: col block j keeps m <= l within block."""
    m = np.arange(128)[:, None]
    l = np.arange(128)[None, :]
    blk = (m <= l).astype(np.float32)
    return np.tile(blk, (1, 4))


def _bcast(ap, n):
    """Append a stride-0 axis of length n to an AP (free-dim broadcast)."""
    return bass.AP(tensor=ap.tensor, offset=ap.offset, ap=list(ap.ap) + [[0, n]])


def _build(alpha: float, reps: int = 1, dbg: bool = False):
    nc = bacc.Bacc(trn_type="TRN2", target_bir_lowering=False, debug=False)

    qkt_d = nc.dram_tensor("qkt", [NP, 128, L], F32R, kind="ExternalInput").ap()
    v_d = nc.dram_tensor("v", [NP, 128, NT, D + 1], BF16, kind="ExternalInput").ap()
    wq_d = nc.dram_tensor("wq", [HPC, D, 128], F32R, kind="ExternalInput").ap()
    wk_d = nc.dram_tensor("wk", [HPC, D, 128], F32R, kind="ExternalInput").ap()
    out_d = nc.dram_tensor("out", [NP, 128, NT, D], F32, kind="ExternalOutput").ap()
    dbg_d = {}
    if dbg:
        for nm, shape, dt in [
            ("dA", [128, 2 * L], BF16), ("dk12n", [128, 6, 128], BF16),
            ("dPsA", [128, 512], BF16), ("dPsB", [128, 512], BF16),
            ("dPsF", [128, 512], BF16), ("dSsb", [128, 3, 65], BF16),
            ("dous", [128, NT, D + 1], F32),
            ("dden", [128, NT], F32), ("ddi", [128, NT], F32),
            ("doutf", [128, NT, D], F32),
        ]:
            dbg_d[nm] = nc.dram_tensor(nm, shape, dt,
                                       kind="ExternalOutput").ap()

    ident_d = nc.inline_tensor(_ident_np(), name="identbf").ap()
    maskbd_d = nc.inline_tensor(_maskbd_np(), name="maskbd").ap()

    with tile.TileContext(nc) as tc:
        with (
            tc.tile_pool(name="statics", bufs=1) as statics,
            tc.tile_pool(name="io", bufs=4) as io,
            tc.tile_pool(name="sb", bufs=3) as sb,
            tc.tile_pool(name="ps_fm", bufs=2, space="PSUM") as ps_fm,
            tc.tile_pool(name="ps_pp", bufs=2, space="PSUM") as ps_pp,
            tc.tile_pool(name="ps_tr", bufs=2, space="PSUM") as ps_tr,
            tc.tile_pool(name="ps_mx", bufs=2, space="PSUM") as ps_mx,
        ):
            ident = statics.tile([128, 128], BF16, tag="ident")
            nc.sync.dma_start(out=ident, in_=ident_d)
            maskbd = statics.tile([128, 512], F32, tag="maskbd")
            nc.sync.dma_start(out=maskbd, in_=maskbd_d)

            wq_t, wk_t = [], []
            for hl in range(HPC):
                t_wq = statics.tile([64, 128], F32R, tag=f"wq{hl}", name=f"wq{hl}")
                nc.sync.dma_start(out=t_wq, in_=wq_d[hl])
                wq_t.append(t_wq)
                t_wk = statics.tile([128, 128], F32R, tag=f"wk{hl}", name=f"wk{hl}")
                nc.sync.dma_start(out=t_wk[64:128, :], in_=wk_d[hl])
                wk_t.append(t_wk)

            # greedy DVE/ACT load balancer for PSUM->SBUF evacuations
            load = {"dve": 0.0, "act": 0.0}

            def evac(dst, src, cols, relu=False):
                cd = cols * 1.04 + 120.0 + load["dve"]
                ca = cols * 0.833 + 293.0 + load["act"]
                if cd <= ca:
                    load["dve"] = cd
                    if relu:
                        nc.vector.tensor_relu(dst, src)
                    else:
                        nc.vector.tensor_copy(dst, src)
                else:
                    load["act"] = ca
                    if relu:
                        nc.scalar.activation(dst, src, AF.Relu)
                    else:
                        nc.scalar.copy(dst, src)

            st = {}  # per-pair live tiles

            def stage_load(p):
                pd = p % NP
                qkT = io.tile([128, L], F32R, tag="qkT", name=f"qkT{p}")
                nc.sync.dma_start(out=qkT, in_=qkt_d[pd])
                vn = io.tile([128, NT, D + 1], BF16, tag="vn", name=f"vn{p}")
                nc.sync.dma_start(out=vn, in_=v_d[pd])
                st[p] = {"qkT": qkT, "vn": vn}

            def stage_fmap(p):
                hl = (p % NP) % HPC
                qkT = st[p]["qkT"]
                A = sb.tile([128, 2 * L], BF16, tag="A", name=f"A{p}")
                ad = sb.tile([128, 2 * L], BF16, tag="ad", name=f"ad{p}")
                tmp = sb.tile([128, 2 * L], BF16, tag="tmp", name=f"tmp{p}")
                for qk in range(2):
                    wm = wq_t[hl] if qk == 0 else wk_t[hl][64:128, :]
                    rows = slice(0, 64) if qk == 0 else slice(64, 128)
                    for lc in range(2):
                        fm = ps_fm.tile([128, 512], F32, tag="fm",
                                        name=f"fm{p}_{qk}_{lc}")
                        nc.tensor.matmul(fm, wm,
                                         qkT[rows, lc * 512:(lc + 1) * 512],
                                         start=True, stop=True)
                        cs = slice(qk * L + lc * 512, qk * L + (lc + 1) * 512)
                        # x1 -> A top; sigmoid(x2pre) -> tmp bottom
                        evac(A[0:64, cs], fm[0:64, :], 512, relu=True)
                        nc.scalar.activation(tmp[64:128, cs], fm[64:128, :],
                                             AF.Sigmoid)
                        nc.gpsimd.dma_start(out=ad[64:128, cs], in_=A[0:64, cs])
                        if qk == 0:
                            # q2 = -alpha * max(sig, 0.5) * q1
                            nc.vector.tensor_scalar(
                                tmp[64:128, cs], tmp[64:128, cs], 0.5, -alpha,
                                op0=OP.max, op1=OP.mult)
                            nc.vector.tensor_tensor(
                                A[64:128, cs], tmp[64:128, cs], ad[64:128, cs],
                                op=OP.mult)
                        else:
                            # k2 = max(sig, 0.5) * k1
                            nc.vector.scalar_tensor_tensor(
                                A[64:128, cs], tmp[64:128, cs], 0.5,
                                ad[64:128, cs], op0=OP.max, op1=OP.mult)
                st[p]["A"] = A

            def Q12(p, c):
                return st[p]["A"][:, c * 128:(c + 1) * 128]

            def K12(p, c):
                return st[p]["A"][:, L + c * 128:L + (c + 1) * 128]

            def stage_prep(p):
                A = st[p]["A"]
                vn = st[p]["vn"]
                # transposes of K12 chunks 0..5 -> k12nat (natural layout)
                k12n = sb.tile([128, 6, 128], BF16, tag="k12n", name=f"k12n{p}")
                trs = []
                for g in range(2):
                    tr = ps_tr.tile([128, 512], BF16, tag="tr",
                                    name=f"tr{p}_{g}")
                    n = 4 if g == 0 else 2
                    for j in range(n):
                        nc.tensor.transpose(tr[:, j * 128:(j + 1) * 128],
                                            K12(p, g * 4 + j), ident)
                    trs.append((tr, n))
                # intra-chunk quadratic blocks
                pTd = ps_pp.tile([128, 512], F32, tag="pp", name=f"pTd{p}")
                for c in range(4):
                    nc.tensor.matmul(pTd[:, c * 128:(c + 1) * 128],
                                     K12(p, c), Q12(p, c),
                                     start=True, stop=True,
                                     skip_group_check=True)
                # evac transposes while diag MMs run
                for g, (tr, n) in enumerate(trs):
                    evac(k12n[:, g * 4:g * 4 + n, :],
                         tr[:, 0:n * 128].rearrange("p (a b) -> p a b", a=n),
                         n * 128)
                pTd2 = ps_pp.tile([128, 512], F32, tag="pp", name=f"pTd2{p}")
                for c in range(4, 8):
                    nc.tensor.matmul(pTd2[:, (c - 4) * 128:(c - 3) * 128],
                                     K12(p, c), Q12(p, c),
                                     start=True, stop=True,
                                     skip_group_check=True)
                PsA = sb.tile([128, 512], BF16, tag="PsA", name=f"PsA{p}")
                nc.vector.tensor_tensor(PsA, pTd, maskbd, op=OP.mult)
                pTf = ps_pp.tile([128, 512], F32, tag="pp", name=f"pTf{p}")
                for j in range(4):
                    nc.tensor.matmul(pTf[:, j * 128:(j + 1) * 128],
                                     K12(p, 2 * j), Q12(p, 2 * j + 1),
                                     start=True, stop=True,
                                     skip_group_check=True)
                PsB = sb.tile([128, 512], BF16, tag="PsB", name=f"PsB{p}")
                nc.vector.tensor_tensor(PsB, pTd2, maskbd, op=OP.mult)
                # state deltas: sd[:, j, :] accumulates super-chunk j
                mx1 = ps_mx.tile([128, 512], F32, tag="mx", name=f"mx1{p}")
                sd = mx1[:, 260:455].rearrange("p (j d) -> p j d", j=3)
                for c in range(6):
                    j = c // 2
                    nc.tensor.matmul(sd[:, j, :], k12n[:, c, :], vn[:, c, :],
                                     start=(c % 2 == 0), stop=(c % 2 == 1),
                                     skip_group_check=True)
                PsF = sb.tile([128, 512], BF16, tag="PsF", name=f"PsF{p}")
                evac(PsF, pTf, 512)
                # S snapshots (bf16): S[0]=sd0; S[j]=S[j-1]+sd[j]
                Ssb = sb.tile([128, 3, 65], BF16, tag="Ssb", name=f"Ssb{p}")
                nc.vector.tensor_copy(Ssb[:, 0, :], sd[:, 0, :])
                for j in (1, 2):
                    nc.vector.tensor_tensor(Ssb[:, j, :], Ssb[:, j - 1, :],
                                            sd[:, j, :], op=OP.add)
                st[p].update(PsA=PsA, PsB=PsB, PsF=PsF, Ssb=Ssb, mx1=mx1,
                             k12n=k12n)

            def stage_out(p):
                pd = p % NP
                s = st[p]
                vn = s["vn"]
                ous = sb.tile([128, NT, D + 1], F32, tag="ous", name=f"ous{p}")
                for g in range(2):
                    if g == 0:
                        ou = s["mx1"][:, 0:260]
                    else:
                        mx2 = ps_mx.tile([128, 512], F32, tag="mx",
                                         name=f"mx2{p}")
                        ou = mx2[:, 0:260]
                    ou = ou.rearrange("p (t d) -> p t d", t=4)
                    for t in range(4):
                        c = g * 4 + t
                        Pst = (s["PsA"] if g == 0 else s["PsB"])
                        mms = [(Pst[:, t * 128:(t + 1) * 128], vn[:, c, :])]
                        if c % 2 == 1:
                            mms.append((s["PsF"][:, (c // 2) * 128:
                                                 (c // 2 + 1) * 128],
                                        vn[:, c - 1, :]))
                        if c >= 2:
                            mms.append((Q12(p, c), s["Ssb"][:, c // 2 - 1, :]))
                        for i, (lhsT, rhs) in enumerate(mms):
                            nc.tensor.matmul(ou[:, t, :], lhsT, rhs,
                                             start=(i == 0),
                                             stop=(i == len(mms) - 1),
                                             skip_group_check=True)
                    evac(ous[:, g * 4:(g + 1) * 4, :], ou, 260)
                if dbg and p == 0:
                    nc.sync.dma_start(out=dbg_d["dA"], in_=s["A"])
                    nc.sync.dma_start(out=dbg_d["dk12n"], in_=s["k12n"])
                    nc.sync.dma_start(out=dbg_d["dPsA"], in_=s["PsA"])
                    nc.sync.dma_start(out=dbg_d["dPsB"], in_=s["PsB"])
                    nc.sync.dma_start(out=dbg_d["dPsF"], in_=s["PsF"])
                    nc.sync.dma_start(out=dbg_d["dSsb"], in_=s["Ssb"])
                    nc.sync.dma_start(out=dbg_d["dous"], in_=ous)
                den = sb.tile([128, NT], F32, tag="den", name=f"den{p}")
                nc.vector.tensor_scalar(den, ous[:, :, 64], EPS, None,
                                        op0=OP.add)
                di = sb.tile([128, NT], F32, tag="di", name=f"di{p}")
                nc.vector.reciprocal(di, den)
                outf = io.tile([128, NT, D], F32, tag="outf", name=f"outf{p}")
                nc.vector.tensor_tensor(outf, ous[:, :, 0:64],
                                        _bcast(di[:, :], 64), op=OP.mult)
                if dbg and p == 0:
                    nc.sync.dma_start(out=dbg_d["dden"], in_=den)
                    nc.sync.dma_start(out=dbg_d["ddi"], in_=di)
                    nc.sync.dma_start(out=dbg_d["doutf"], in_=outf)
                nc.sync.dma_start(out=out_d[pd], in_=outf)
                del st[p]

            seq = list(range(reps * NP))
            n = len(seq)
            for j in range(min(3, n)):
                stage_load(seq[j])
            stage_fmap(seq[0])
            if n > 1:
                stage_fmap(seq[1])
            for i, p in enumerate(seq):
                stage_prep(p)
                if i + 2 < n:
                    stage_fmap(seq[i + 2])
                stage_out(p)
                if i + 3 < n:
                    stage_load(seq[i + 3])
    nc.compile()
    return nc


def _get_nc(alpha: float = 0.3, reps: int = 1):
    key = ("nc", float(alpha), reps)
    if key not in _CACHE:
        _CACHE[key] = _build(float(alpha), reps)
    return _CACHE[key]


def prepare_in_maps(inputs: dict) -> list[dict]:
    """Host-side shard + relayout: full inputs -> per-core in_maps."""
    q = np.ascontiguousarray(np.asarray(inputs["query_states"], np.float32))
    k = np.ascontiguousarray(np.asarray(inputs["key_states"], np.float32))
    v = np.ascontiguousarray(np.asarray(inputs["value_states"], np.float32))
    v = np.concatenate([v, np.ones(v.shape[:-1] + (1,), np.float32)], axis=-1)
    w1q = np.asarray(inputs["W1q"], np.float32)
    w1k = np.asarray(inputs["W1k"], np.float32)
    w2q = np.asarray(inputs["W2q"], np.float32)
    w2k = np.asarray(inputs["W2k"], np.float32)
    in_maps = []
    for c in range(NCORES):
        hs = slice(c * HPC, (c + 1) * HPC)
        qc = q[:, hs].reshape(NP, L, D).transpose(0, 2, 1)
        kc = k[:, hs].reshape(NP, L, D).transpose(0, 2, 1)
        qkt = np.ascontiguousarray(np.concatenate([qc, kc], axis=1))
        vc = v[:, hs].reshape(NP, NT, 128, D + 1).transpose(0, 2, 1, 3)
        vc = np.ascontiguousarray(vc).astype(BF)
        wq = np.ascontiguousarray(np.concatenate([w1q[hs], w2q[hs]], axis=2))
        wk = np.ascontiguousarray(np.concatenate([w1k[hs], w2k[hs]], axis=2))
        in_maps.append({"qkt": qkt, "v": vc, "wq": wq, "wk": wk})
    return in_maps


def finish_out(res_out: np.ndarray) -> np.ndarray:
    """Device-tiled [NP, 128, NT, D] -> [NP, L, D]."""
    return res_out.transpose(0, 2, 1, 3).reshape(NP, L, D)


def kernel(query_states, key_states, value_states, W1q, W1k, W2q, W2k, alpha):
    al = float(np.asarray(alpha, dtype=np.float32).reshape(-1)[0])
    inputs = {
        "query_states": query_states, "key_states": key_states,
        "value_states": value_states, "W1q": W1q, "W1k": W1k,
        "W2q": W2q, "W2k": W2k,
    }
    in_maps = prepare_in_maps(inputs)
    nc = _get_nc(al)
    res = run_bass_kernel_spmd(nc, in_maps, core_ids=list(range(NCORES)))
    out = np.empty((B, H, L, D), dtype=np.float32)
    for c in range(NCORES):
        o = finish_out(res.results[c]["out"]).reshape(B, HPC, L, D)
        out[:, c * HPC:(c + 1) * HPC] = o
    return out
